# revision 26
# baseline (speedup 1.0000x reference)
"""Trainium2 Bass kernel for nn_EntanglementRegularizer (histogram_binning).

Math: the reference computes entropy of hist_j = mean_i softmax_j(-2(y_i-b_j)^2).
The softmax denominator Z(y) = sum_j exp(-2(y-b_j)^2) is a theta function that
is constant to machine precision for |y| <= 6 (bins span [-10,10], sigma=0.5
>> bin spacing), so hist_j is proportional to the Gaussian KDE
u_j = sum_i exp(-2(y_i-b_j)^2) and the normalization cancels.

Kernel: split sigma^2 = sigma1^2 + sigma2^2, sigma1 = sigma2 = 0.5/sqrt(2):
  stage 1 (on device, per core, data-parallel over N):
      v(g) = sum_i exp(-4 (y_i - g)^2) on an M-point coarse grid
      -> M ACTIVATE instructions: Derivative_Erf(2*y + bias_g) with
         free-dim accumulation (accum_out); partition reduction via a
         ones-stationary matmul.
  all-gather v across the 8 cores (160-byte collective), local 8-way sum
  stage 2 (tiny): u = v @ W2 with W2[g,j] = exp(-4 (b_j - g)^2), then
      p = u/sum(u), out = 0.01 * sum(p * ln(p + 1e-10)), on every core.

The trapezoid error of the sigma-split is ~2*exp(-2*pi^2*(sigma/2/h)^2) per
element; it oscillates in y so it averages out against the smooth data
density; M=32 measured ~1e-7 relative error end-to-end (f32 floor ~2e-6).
"""

import numpy as np

NCORES = 8
P = 128  # SBUF partitions
M = 26  # coarse KDE grid points (cliff below 26; numpy 1.6e-6 here)
NBINS = 256
GRID_LO, GRID_HI = -8.0, 8.0
N_TOTAL = 8 * 16 * 128 * 128  # 2,097,152 elements (8,16,128,128) f32
F = N_TOTAL // (NCORES * P)  # 2048 free-dim elements per partition per core

# If True, the 8 cores all-gather their partial KDE vectors and every core
# computes the final entropy on device (host just reads core 0's scalar).
# If False, each core returns its M partial sums and the host does the
# 8-way sum + 256-bin entropy (the gather/unshard step) in float64.
DEVICE_REDUCE = True

_COMPILED = {}


def _build_program(device_reduce):
    import concourse.bacc as bacc
    import concourse.mybir as mybir
    import concourse.tile as tile

    f32 = mybir.dt.float32
    nc = bacc.Bacc("TRN2", target_bir_lowering=False, debug=False, num_devices=NCORES)

    y_d = nc.dram_tensor("y", [P, F], f32, kind="ExternalInput")
    bias_d = nc.dram_tensor("bias", [P, M], f32, kind="ExternalInput")
    ones_d = nc.dram_tensor("ones", [P, 1], f32, kind="ExternalInput")
    if device_reduce:
        w2_d = nc.dram_tensor("w2", [M, NBINS], f32, kind="ExternalInput")
        out_d = nc.dram_tensor("out", [1, 1], f32, kind="ExternalOutput")
    else:
        out_d = nc.dram_tensor("out", [1, M], f32, kind="ExternalOutput")

    DERF = mybir.ActivationFunctionType.Derivative_Erf
    LN = mybir.ActivationFunctionType.Ln
    X = mybir.AxisListType.X

    with tile.TileContext(nc) as tc:
        with (
            tc.tile_pool(name="sbuf", bufs=1) as pool,
            tc.tile_pool(name="psum", bufs=1, space="PSUM") as psum,
            tc.tile_pool(name="dram", bufs=1, space="DRAM") as dram,
        ):
            y_sb = pool.tile([P, F], f32, tag="y")
            bias_sb = pool.tile([P, M], f32, tag="bias")
            ones_sb = pool.tile([P, 1], f32, tag="ones")
            acc_sb = pool.tile([P, M], f32, tag="acc")

            # split the 1 MiB input load across a few DMA issues (each fans
            # out across the 16 HW DMA engines; issue cost ~0.6us, serialized
            # per issuing engine)
            nsplit = 2
            cw = F // nsplit
            for i in range(nsplit):
                sl = slice(i * cw, (i + 1) * cw)
                nc.sync.dma_start(y_sb[:, sl], y_d[:, sl])
            nc.gpsimd.dma_start(bias_sb[:], bias_d[:])
            nc.gpsimd.dma_start(ones_sb[:], ones_d[:])
            if device_reduce:
                w2_sb = pool.tile([M, NBINS], f32, tag="w2")
                nc.gpsimd.dma_start(w2_sb[:], w2_d[:])

            # preload the Derivative_Erf LUT while the y DMA is in flight so
            # the first real ACT instruction doesn't pay the table switch
            warm_sb = pool.tile([1, 1], f32, tag="warm")
            nc.vector.memset(warm_sb[:], 0.0)
            nc.scalar.activation(warm_sb[:], warm_sb[:], DERF, bias=warm_sb[:], scale=1.0)

            if device_reduce:
                # warm up the ncfw collective path during the ACT phase: a
                # dummy 32-byte all-gather absorbs the ~13us trigger latency
                wcc_in = dram.tile([1, 1], f32, tag="wcc_in")
                wcc_out = dram.tile([NCORES, 1], f32, tag="wcc_out")
                nc.sync.dma_start(wcc_in[:], warm_sb[:])
                nc.gpsimd.collective_compute(
                    "AllGather",
                    mybir.AluOpType.bypass,
                    replica_groups=[list(range(NCORES))],
                    ins=[wcc_in.opt()],
                    outs=[wcc_out.opt()],
                )

            # stage 1: per-grid-point Gaussian sums over this core's shard
            with tc.tile_pool(name="escratch", bufs=2) as epool:
                for r in range(M):
                    e_sb = epool.tile([P, F], f32, tag="e")
                    nc.scalar.activation(
                        e_sb[:],
                        y_sb[:],
                        DERF,
                        bias=bias_sb[:, r : r + 1],
                        scale=2.0,
                        accum_out=acc_sb[:, r : r + 1],
                    )

            # partition reduction: v[1, M] = ones[P,1].T @ acc[P, M]
            v_ps = psum.tile([1, M], f32, tag="v")
            nc.tensor.matmul(v_ps[:], ones_sb[:], acc_sb[:])
            v_sb = pool.tile([1, M], f32, tag="v_sb")
            nc.vector.tensor_copy(v_sb[:], v_ps[:])

            if not device_reduce:
                nc.sync.dma_start(out_d[:], v_sb[:])
            else:
                # all-gather the M partial sums across the 8 cores (one ring
                # phase - cheaper than AllReduce), then sum locally.
                cc_in = dram.tile([1, M], f32, tag="cc_in")
                cc_out = dram.tile([NCORES, M], f32, tag="cc_out")
                nc.sync.dma_start(cc_in[:], v_sb[:])
                nc.gpsimd.collective_compute(
                    "AllGather",
                    mybir.AluOpType.bypass,
                    replica_groups=[list(range(NCORES))],
                    ins=[cc_in.opt()],
                    outs=[cc_out.opt()],
                )
                # load as [M partitions, NCORES] and reduce over free dim
                vg_sb = pool.tile([M, NCORES], f32, tag="vg")
                nc.sync.dma_start(vg_sb[:], cc_out.opt().rearrange("c m -> m c"))
                v_col = pool.tile([M, 1], f32, tag="v_col")
                nc.vector.reduce_sum(v_col[:], vg_sb[:], axis=X)

                # stage 2: u[1, NBINS] = v_col.T @ W2
                u_ps = psum.tile([1, NBINS], f32, tag="u")
                nc.tensor.matmul(u_ps[:], v_col[:], w2_sb[:])
                u_sb = pool.tile([1, NBINS], f32, tag="u_sb")
                nc.vector.tensor_copy(u_sb[:], u_ps[:])

                # p = u / sum(u); out = 0.01 * sum(p * ln(p + 1e-10))
                s_sb = pool.tile([1, 1], f32, tag="s")
                nc.vector.reduce_sum(s_sb[:], u_sb[:], axis=X)
                rcp_sb = pool.tile([1, 1], f32, tag="rcp")
                nc.vector.reciprocal(rcp_sb[:], s_sb[:])
                p_sb = pool.tile([1, NBINS], f32, tag="p")
                nc.vector.tensor_scalar_mul(p_sb[:], u_sb[:], rcp_sb[:])
                eps_sb = pool.tile([1, 1], f32, tag="eps")
                nc.vector.memset(eps_sb[:], 1e-10)
                l_sb = pool.tile([1, NBINS], f32, tag="l")
                nc.scalar.activation(l_sb[:], p_sb[:], LN, bias=eps_sb[:], scale=1.0)
                pl_sb = pool.tile([1, NBINS], f32, tag="pl")
                nc.vector.tensor_mul(pl_sb[:], p_sb[:], l_sb[:])
                h_sb = pool.tile([1, 1], f32, tag="h")
                nc.vector.reduce_sum(h_sb[:], pl_sb[:], axis=X)
                o_sb = pool.tile([1, 1], f32, tag="o")
                nc.scalar.mul(o_sb[:], h_sb[:], 0.01)
                nc.sync.dma_start(out_d[:], o_sb[:])

    nc.compile()
    return nc


def _get_program(device_reduce):
    key = ("nc", device_reduce)
    if key not in _COMPILED:
        _COMPILED[key] = _build_program(device_reduce)
    return _COMPILED[key]


def _grid():
    return np.linspace(GRID_LO, GRID_HI, M, dtype=np.float64)


def _host_inputs(y_hat, bins, device_reduce):
    y = np.ascontiguousarray(np.asarray(y_hat, dtype=np.float32).reshape(-1))
    assert y.size == N_TOTAL, y.size
    shards = y.reshape(NCORES, P, F)

    grid = _grid()
    bias_np = np.broadcast_to((-2.0 * grid).astype(np.float32)[None, :], (P, M)).copy()
    ones_np = np.ones((P, 1), dtype=np.float32)

    maps = []
    for i in range(NCORES):
        m = {
            "y": np.ascontiguousarray(shards[i]),
            "bias": bias_np,
            "ones": ones_np,
        }
        if device_reduce:
            binsf = np.asarray(bins, dtype=np.float64).reshape(-1)
            m["w2"] = np.exp(-4.0 * (binsf[None, :] - grid[:, None]) ** 2).astype(
                np.float32
            )
        maps.append(m)
    return maps


def run(y_hat, bins, device_reduce=None, **spmd_kwargs):
    """Build + run on the 8 cores; returns (scalar_output, BassKernelResults)."""
    from concourse import bass_utils

    if device_reduce is None:
        device_reduce = DEVICE_REDUCE
    nc = _get_program(device_reduce)
    in_maps = _host_inputs(y_hat, bins, device_reduce)
    res = bass_utils.run_bass_kernel_spmd(
        nc, in_maps, core_ids=list(range(NCORES)), **spmd_kwargs
    )
    if device_reduce:
        out = np.asarray(res.results[0]["out"], dtype=np.float32).reshape(())
    else:
        # gather/unshard: sum the per-core partial KDE vectors, then the
        # (tiny) stage-2 interpolation + entropy in float64 on host
        v = np.zeros(M, dtype=np.float64)
        for r in res.results:
            v += np.asarray(r["out"], dtype=np.float64).reshape(-1)
        grid = _grid()
        binsf = np.asarray(bins, dtype=np.float64).reshape(-1)
        w2 = np.exp(-4.0 * (binsf[None, :] - grid[:, None]) ** 2)
        u = v @ w2
        p = u / u.sum()
        out = np.float32(0.01 * (p * np.log(p + 1e-10)).sum()).reshape(())[()]
        out = np.asarray(out, dtype=np.float32).reshape(())
    return out, res


def kernel(y_hat, bins):
    out, _ = run(y_hat, bins)
    return out


# revision 31
# speedup vs baseline: 1.0218x; 1.0218x over previous
"""Trainium2 Bass kernel for nn_EntanglementRegularizer (histogram_binning).

Math: the reference computes entropy of hist_j = mean_i softmax_j(-2(y_i-b_j)^2).
The softmax denominator Z(y) = sum_j exp(-2(y-b_j)^2) is a theta function that
is constant to machine precision for |y| <= 6 (bins span [-10,10], sigma=0.5
>> bin spacing), so hist_j is proportional to the Gaussian KDE
u_j = sum_i exp(-2(y_i-b_j)^2) and the normalization cancels.

Kernel: split sigma^2 = sigma1^2 + sigma2^2, sigma1 = sigma2 = 0.5/sqrt(2):
  stage 1 (on device, per core, data-parallel over N):
      v(g) = sum_i exp(-4 (y_i - g)^2) on an M-point coarse grid
      -> M ACTIVATE instructions: Derivative_Erf(2*y + bias_g) with
         free-dim accumulation (accum_out); partition reduction via a
         ones-stationary matmul.
  all-gather v across the 8 cores (160-byte collective), local 8-way sum
  stage 2 (tiny): u = v @ W2 with W2[g,j] = exp(-4 (b_j - g)^2), then
      p = u/sum(u), out = 0.01 * sum(p * ln(p + 1e-10)), on every core.

The trapezoid error of the sigma-split is ~2*exp(-2*pi^2*(sigma/2/h)^2) per
element; it oscillates in y so it averages out against the smooth data
density; M=32 measured ~1e-7 relative error end-to-end (f32 floor ~2e-6).
"""

import numpy as np

NCORES = 8
P = 128  # SBUF partitions
M = 26  # coarse KDE grid points (cliff below 26; numpy 1.6e-6 here)
NBINS = 256
GRID_LO, GRID_HI = -8.0, 8.0
N_TOTAL = 8 * 16 * 128 * 128  # 2,097,152 elements (8,16,128,128) f32
F = N_TOTAL // (NCORES * P)  # 2048 free-dim elements per partition per core

# If True, the 8 cores all-gather their partial KDE vectors and every core
# computes the final entropy on device (host just reads core 0's scalar).
# If False, each core returns its M partial sums and the host does the
# 8-way sum + 256-bin entropy (the gather/unshard step) in float64.
DEVICE_REDUCE = True

_COMPILED = {}


def _build_program(device_reduce):
    import concourse.bacc as bacc
    import concourse.mybir as mybir
    import concourse.tile as tile

    f32 = mybir.dt.float32
    nc = bacc.Bacc("TRN2", target_bir_lowering=False, debug=False, num_devices=NCORES)

    y_d = nc.dram_tensor("y", [P, F], f32, kind="ExternalInput")
    bias_d = nc.dram_tensor("bias", [P, M], f32, kind="ExternalInput")
    ones_d = nc.dram_tensor("ones", [P, 1], f32, kind="ExternalInput")
    if device_reduce:
        w2_d = nc.dram_tensor("w2", [M, NBINS], f32, kind="ExternalInput")
        out_d = nc.dram_tensor("out", [1, 1], f32, kind="ExternalOutput")
    else:
        out_d = nc.dram_tensor("out", [1, M], f32, kind="ExternalOutput")

    DERF = mybir.ActivationFunctionType.Derivative_Erf
    LN = mybir.ActivationFunctionType.Ln
    X = mybir.AxisListType.X

    with tile.TileContext(nc) as tc:
        with (
            tc.tile_pool(name="sbuf", bufs=1) as pool,
            tc.tile_pool(name="psum", bufs=1, space="PSUM") as psum,
            tc.tile_pool(name="dram", bufs=1, space="DRAM") as dram,
        ):
            y_sb = pool.tile([P, F], f32, tag="y")
            bias_sb = pool.tile([P, M], f32, tag="bias")
            ones_sb = pool.tile([P, 1], f32, tag="ones")
            acc_sb = pool.tile([P, M], f32, tag="acc")

            # split the 1 MiB input load across a few DMA issues (each fans
            # out across the 16 HW DMA engines; issue cost ~0.6us, serialized
            # per issuing engine)
            nsplit = 4
            cw = F // nsplit
            for i in range(nsplit):
                sl = slice(i * cw, (i + 1) * cw)
                nc.sync.dma_start(y_sb[:, sl], y_d[:, sl])
            nc.gpsimd.dma_start(bias_sb[:], bias_d[:])
            nc.gpsimd.dma_start(ones_sb[:], ones_d[:])
            if device_reduce:
                w2_sb = pool.tile([M, NBINS], f32, tag="w2")
                nc.gpsimd.dma_start(w2_sb[:], w2_d[:])

            # preload the Derivative_Erf LUT while the y DMA is in flight so
            # the first real ACT instruction doesn't pay the table switch
            warm_sb = pool.tile([1, 1], f32, tag="warm")
            nc.vector.memset(warm_sb[:], 0.0)
            nc.scalar.activation(warm_sb[:], warm_sb[:], DERF, bias=warm_sb[:], scale=1.0)

            if device_reduce:
                # warm up the ncfw collective path during the ACT phase: a
                # dummy 32-byte all-gather absorbs the ~13us trigger latency
                wcc_in = dram.tile([1, 1], f32, tag="wcc_in")
                wcc_out = dram.tile([NCORES, 1], f32, tag="wcc_out")
                nc.sync.dma_start(wcc_in[:], warm_sb[:])
                nc.gpsimd.collective_compute(
                    "AllGather",
                    mybir.AluOpType.bypass,
                    replica_groups=[list(range(NCORES))],
                    ins=[wcc_in.opt()],
                    outs=[wcc_out.opt()],
                )

            # stage 1: per-grid-point Gaussian sums over this core's shard
            with tc.tile_pool(name="escratch", bufs=2) as epool:
                for r in range(M):
                    e_sb = epool.tile([P, F], f32, tag="e")
                    nc.scalar.activation(
                        e_sb[:],
                        y_sb[:],
                        DERF,
                        bias=bias_sb[:, r : r + 1],
                        scale=2.0,
                        accum_out=acc_sb[:, r : r + 1],
                    )

            # partition reduction: v[1, M] = ones[P,1].T @ acc[P, M]
            v_ps = psum.tile([1, M], f32, tag="v")
            nc.tensor.matmul(v_ps[:], ones_sb[:], acc_sb[:])
            v_sb = pool.tile([1, M], f32, tag="v_sb")
            nc.vector.tensor_copy(v_sb[:], v_ps[:])

            if not device_reduce:
                nc.sync.dma_start(out_d[:], v_sb[:])
            else:
                # all-gather the M partial sums across the 8 cores (one ring
                # phase - cheaper than AllReduce), then sum locally.
                cc_in = dram.tile([1, M], f32, tag="cc_in")
                cc_out = dram.tile([NCORES, M], f32, tag="cc_out")
                nc.sync.dma_start(cc_in[:], v_sb[:])
                nc.gpsimd.collective_compute(
                    "AllGather",
                    mybir.AluOpType.bypass,
                    replica_groups=[list(range(NCORES))],
                    ins=[cc_in.opt()],
                    outs=[cc_out.opt()],
                )
                # load as [M partitions, NCORES] and reduce over free dim
                vg_sb = pool.tile([M, NCORES], f32, tag="vg")
                nc.sync.dma_start(vg_sb[:], cc_out.opt().rearrange("c m -> m c"))
                v_col = pool.tile([M, 1], f32, tag="v_col")
                nc.vector.reduce_sum(v_col[:], vg_sb[:], axis=X)

                # stage 2: u[1, NBINS] = v_col.T @ W2
                u_ps = psum.tile([1, NBINS], f32, tag="u")
                nc.tensor.matmul(u_ps[:], v_col[:], w2_sb[:])
                u_sb = pool.tile([1, NBINS], f32, tag="u_sb")
                nc.vector.tensor_copy(u_sb[:], u_ps[:])

                # p = u / sum(u); out = 0.01 * sum(p * ln(p + 1e-10))
                s_sb = pool.tile([1, 1], f32, tag="s")
                nc.vector.reduce_sum(s_sb[:], u_sb[:], axis=X)
                rcp_sb = pool.tile([1, 1], f32, tag="rcp")
                nc.vector.reciprocal(rcp_sb[:], s_sb[:])
                p_sb = pool.tile([1, NBINS], f32, tag="p")
                nc.vector.tensor_scalar_mul(p_sb[:], u_sb[:], rcp_sb[:])
                eps_sb = pool.tile([1, 1], f32, tag="eps")
                nc.vector.memset(eps_sb[:], 1e-10)
                l_sb = pool.tile([1, NBINS], f32, tag="l")
                nc.scalar.activation(l_sb[:], p_sb[:], LN, bias=eps_sb[:], scale=1.0)
                pl_sb = pool.tile([1, NBINS], f32, tag="pl")
                nc.vector.tensor_mul(pl_sb[:], p_sb[:], l_sb[:])
                h_sb = pool.tile([1, 1], f32, tag="h")
                nc.vector.reduce_sum(h_sb[:], pl_sb[:], axis=X)
                o_sb = pool.tile([1, 1], f32, tag="o")
                nc.scalar.mul(o_sb[:], h_sb[:], 0.01)
                nc.sync.dma_start(out_d[:], o_sb[:])

    nc.compile()
    return nc


def _get_program(device_reduce):
    key = ("nc", device_reduce)
    if key not in _COMPILED:
        _COMPILED[key] = _build_program(device_reduce)
    return _COMPILED[key]


def _grid():
    return np.linspace(GRID_LO, GRID_HI, M, dtype=np.float64)


def _host_inputs(y_hat, bins, device_reduce):
    y = np.ascontiguousarray(np.asarray(y_hat, dtype=np.float32).reshape(-1))
    assert y.size == N_TOTAL, y.size
    shards = y.reshape(NCORES, P, F)

    grid = _grid()
    bias_np = np.broadcast_to((-2.0 * grid).astype(np.float32)[None, :], (P, M)).copy()
    ones_np = np.ones((P, 1), dtype=np.float32)

    maps = []
    for i in range(NCORES):
        m = {
            "y": np.ascontiguousarray(shards[i]),
            "bias": bias_np,
            "ones": ones_np,
        }
        if device_reduce:
            binsf = np.asarray(bins, dtype=np.float64).reshape(-1)
            m["w2"] = np.exp(-4.0 * (binsf[None, :] - grid[:, None]) ** 2).astype(
                np.float32
            )
        maps.append(m)
    return maps


def run(y_hat, bins, device_reduce=None, **spmd_kwargs):
    """Build + run on the 8 cores; returns (scalar_output, BassKernelResults)."""
    from concourse import bass_utils

    if device_reduce is None:
        device_reduce = DEVICE_REDUCE
    nc = _get_program(device_reduce)
    in_maps = _host_inputs(y_hat, bins, device_reduce)
    res = bass_utils.run_bass_kernel_spmd(
        nc, in_maps, core_ids=list(range(NCORES)), **spmd_kwargs
    )
    if device_reduce:
        out = np.asarray(res.results[0]["out"], dtype=np.float32).reshape(())
    else:
        # gather/unshard: sum the per-core partial KDE vectors, then the
        # (tiny) stage-2 interpolation + entropy in float64 on host
        v = np.zeros(M, dtype=np.float64)
        for r in res.results:
            v += np.asarray(r["out"], dtype=np.float64).reshape(-1)
        grid = _grid()
        binsf = np.asarray(bins, dtype=np.float64).reshape(-1)
        w2 = np.exp(-4.0 * (binsf[None, :] - grid[:, None]) ** 2)
        u = v @ w2
        p = u / u.sum()
        out = np.float32(0.01 * (p * np.log(p + 1e-10)).sum()).reshape(())[()]
        out = np.asarray(out, dtype=np.float32).reshape(())
    return out, res


def kernel(y_hat, bins):
    out, _ = run(y_hat, bins)
    return out
